# revision 1
# baseline (speedup 1.0000x reference)
"""Trainium2 Bass kernel for nn_Attention_69801808495308.

Softmax-free attention: attn = cos_w*cossim + cov_w*cov/d + var_w*varprod/d is
linear in k-side summaries, so attn @ f_v reassociates into per-head 64x64
matrices (linear-attention trick) - no NxN score matrix is materialized.

Per (group g, head h), with fk/fv/fq the projected features:
  M1 = (fk/||fk||)^T fv_true        [64,64]
  M2 = (fk - mean(fk))^T fv_true    [64,64]   (columns sum to 0 -> q-centering free)
  m3 = kvar^T fv_true               [64]
  out = [cos_w*(fq/||fq||)@M1 + (cov_w/d)*fq_true@M2] @ woT
        + qvar @ RW + b_out,   RW = (var_w/d)*blockdiag(m3) @ woT

Sharding: 8 cores = (group g in 0..3) x (row half s in 0..1); q and k/v rows
are split across the pair. B and RW are linear in the k/v summaries, so each
core computes them on its partial sums and a single pair-wise fp16 AllReduce
(~140KB) finishes them - the only cross-core communication; its latency hides
under the second half of the q-side projections.

Implementation notes:
- All matmul operands fp16 (1 cyc/row on PE, ample mantissa for tol 2e-2);
  PSUM f32; casts happen during SWDGE DMA loads and PSUM evacuations.
- LayerNorm folded: x is centered in SBUF pre-transpose as (mean - x) on the
  Scalar engine; the global sign flip is cancelled by negating w_out on the host
  (beta and b_out must be 0, asserted). Per-token 1/sigma scales are absorbed
  into the U-tensor builds (the cosine term is scale-invariant).
- k/q tiles transposed on the PE (fp16, keeps PE dense/warm); v tiles via the
  serialized hardware DMA-transpose queue in parallel.
- PSUM accumulation obeys the per-bank rule: one open accumulation chain per
  bank at a time (hardware has_written tracking is bank-level).
"""
import numpy as np
from contextlib import ExitStack

import concourse.bass as bass
from concourse import bacc
import concourse.tile as tile
import concourse.mybir as mybir
from concourse.bass_utils import run_bass_kernel_spmd
from concourse.masks import make_identity

f32 = mybir.dt.float32
fp16 = mybir.dt.float16
ALU = mybir.AluOpType
ACTF = mybir.ActivationFunctionType
AXX = mybir.AxisListType.X

QG, N, D = 4, 2048, 512
H, HD = 8, 64
P = 128
LN_EPS = 1e-5
TQ, TK = N // 2, N // 2
QT, KT = TQ // P, TK // P
NCORES = 8


def build_kernel(cos_w, cov_w, var_w):
    c_cov = cov_w / HD
    c_var = var_w / HD

    nc = bacc.Bacc("TRN2", target_bir_lowering=False, debug=False,
                   num_devices=NCORES)
    xq = nc.declare_dram_parameter("xq", [TQ, D], f32, isOutput=False)
    xk = nc.declare_dram_parameter("xk", [TK, D], f32, isOutput=False)
    xv = nc.declare_dram_parameter("xv", [TK, D], f32, isOutput=False)
    wgT_d = nc.declare_dram_parameter("wgT", [D, D], f32, isOutput=False)
    woT_d = nc.declare_dram_parameter("woT", [D, D], f32, isOutput=False)
    out_d = nc.declare_dram_parameter("out", [TQ, D], f32, isOutput=True)

    with tile.TileContext(nc) as tc, ExitStack() as ctx:
        cp = ctx.enter_context(tc.tile_pool(name="cp", bufs=1))
        xp = ctx.enter_context(tc.tile_pool(name="xp", bufs=6))
        slp = ctx.enter_context(tc.tile_pool(name="slp", bufs=4))
        sp = ctx.enter_context(tc.tile_pool(name="sp", bufs=6))
        uqp = ctx.enter_context(tc.tile_pool(name="uqp", bufs=3))
        evp = ctx.enter_context(tc.tile_pool(name="evp", bufs=3))
        psF = ctx.enter_context(tc.tile_pool(name="psF", bufs=3, space="PSUM"))
        psT = ctx.enter_context(tc.tile_pool(name="psT", bufs=3, space="PSUM"))
        psM = ctx.enter_context(tc.tile_pool(name="psM", bufs=1, space="PSUM"))
        psR = ctx.enter_context(tc.tile_pool(name="psR", bufs=1, space="PSUM"))

        # ---- constants (identity built later, after the first loads) ----
        ident16 = cp.tile([P, P], fp16)
        eps_b = cp.tile([P, 1], f32)
        nc.vector.memset(eps_b[:], LN_EPS)

        # ---- persistent state ----
        fk_all = cp.tile([P, KT, D], fp16)     # raw projected k (PSUM units)
        fv_all = cp.tile([P, KT, D], fp16)     # raw projected v
        fq_all = cp.tile([P, QT, D], fp16)     # raw projected q
        uk_all = cp.tile([P, KT, H, 2, HD], fp16)
        st2_k = cp.tile([P, KT, 2], f32)
        st2_v = cp.tile([P, KT, 2], f32)
        st2_q = cp.tile([P, QT, 2], f32)
        ksum = cp.tile([P, KT, H], fp16)
        ksq = cp.tile([P, KT, H], fp16)
        qsum = cp.tile([P, QT, H], fp16)
        qsq = cp.tile([P, QT, H], fp16)
        uq_all = cp.tile([P, QT, H, 2, HD], fp16)

        def stage1(x_d, t, st2_all):
            """Load+cast tile t, LN stats, center on ACT as (mean - x): a global
            sign flip on every feature, cancelled by negating w_out on the host
            (M1/M2 are sign-invariant since k and v flip together; m3/attn flip,
            out-proj flips back)."""
            xt = xp.tile([P, D], fp16, tag="xt")
            nc.gpsimd.dma_start(xt[:], x_d[t * P:(t + 1) * P, :])
            st6 = sp.tile([P, 6], f32, tag="st6")
            nc.vector.bn_stats(st6[:], xt[:])
            nc.vector.bn_aggr(st2_all[:, t, :], st6[:])
            nc.scalar.activation(xt[:], xt[:], ACTF.Identity,
                                 bias=st2_all[:, t, 0:1], scale=-1.0)
            return xt

        def stage2(xt, t, f_dst, head_stats, pe_transpose, evac_scale=None):
            """Transpose (PE or DMA), 4-matmul projection chain, ACT evac
            (optionally scaled); per-head sums on DVE so loads never queue."""
            slab = slp.tile([P, 4, P], fp16, tag="slab")
            if pe_transpose:
                for c in range(4):
                    pt = psT.tile([P, P], fp16, tag="ptx")
                    nc.tensor.transpose(pt[:], xt[:, c * P:(c + 1) * P], ident16[:])
                    if c % 2 == 0:
                        nc.scalar.copy(slab[:, c, :], pt[:])
                    else:
                        nc.vector.tensor_copy(slab[:, c, :], pt[:])
            else:
                nc.sync.dma_start_transpose(slab[:], xt[:])

            psf = psF.tile([P, D], f32, tag="pf")
            for c in range(4):
                nc.tensor.matmul(psf[:], slab[:, c, :], wgT_sb[:, c, :],
                                 start=(c == 0), stop=(c == 3))
            if evac_scale is not None:
                nc.scalar.activation(f_dst[:, t, :], psf[:], ACTF.Copy,
                                     scale=evac_scale)
            else:
                nc.scalar.copy(f_dst[:, t, :], psf[:])
            if head_stats is not None:
                hsum, hsq = head_stats
                fv_ = f_dst[:, t, :].rearrange("p (h d) -> p h d", h=H)
                with nc.allow_low_precision(reason="head sums fit fp16"):
                    nc.vector.reduce_sum(hsum[:, t, :], fv_, axis=AXX)
                    sq = evp.tile([P, D], fp16, tag="sq")
                    nc.vector.tensor_mul(sq[:], f_dst[:, t, :], f_dst[:, t, :])
                    nc.vector.reduce_sum(
                        hsq[:, t, :], sq[:].rearrange("p (h d) -> p h d", h=H),
                        axis=AXX)

        def proj_tile(x_d, t, st2_all, f_dst, head_stats, pe_transpose):
            stage2(stage1(x_d, t, st2_all), t, f_dst, head_stats, pe_transpose)

        # k/v interleaved so the M-chain inputs complete as early as possible;
        # all q-side work is emitted after the AllReduce is issued, filling its
        # latency window.
        # Prefetch the first two tile-pairs BEFORE the weight loads so the
        # x DMAs lead the GpSimd/SWDGE queue; wgT lands just before the first
        # projection needs it, woT (only needed ~100us in) loads after the loop.
        pre = {}
        for t in range(2):
            pre[t] = (stage1(xv, t, st2_v), stage1(xk, t, st2_k))

        make_identity(nc, ident16)
        wgT_sb = cp.tile([P, 4, D], fp16)
        nc.gpsimd.dma_start(wgT_sb[:], wgT_d[:].rearrange("(c p) n -> p c n", p=P))
        bdmask = cp.tile([H, 512], f32)
        nc.gpsimd.memset(bdmask[:], 0.0)
        nc.gpsimd.affine_select(
            out=bdmask[:].rearrange("p (b d) -> p b d", b=H),
            in_=bdmask[:].rearrange("p (b d) -> p b d", b=H),
            compare_op=ALU.not_equal, fill=1.0, base=0,
            pattern=[[-1, H], [0, HD]], channel_multiplier=1)

        for t in range(KT):
            if t in pre:
                xv_t, xk_t = pre[t]
            else:
                xv_t = stage1(xv, t, st2_v)
                xk_t = stage1(xk, t, st2_k)
            inv_sv_t = sp.tile([P, 1], f32, tag="invsv")
            nc.scalar.activation(inv_sv_t[:], st2_v[:, t, 1:2],
                                 ACTF.Abs_reciprocal_sqrt, bias=eps_b[:])
            stage2(xv_t, t, fv_all, None, False, evac_scale=inv_sv_t[:])
            stage2(xk_t, t, fk_all, (ksum, ksq), True)

        woT_sb = cp.tile([P, 4, D], fp16)
        nc.gpsimd.dma_start(woT_sb[:], woT_d[:].rearrange("(c p) n -> p c n", p=P))

        # first half of the q tiles keeps the PE busy while DVE builds U_k
        for t in range(QT // 2):
            proj_tile(xq, t, st2_q, fq_all, (qsum, qsq), True)

        # ---- batched scalar derivations (k/v) ----
        inv_sk = cp.tile([P, KT], f32)
        nc.scalar.activation(inv_sk[:], st2_k[:, :, 1], ACTF.Abs_reciprocal_sqrt,
                             bias=eps_b[:])
        invn_k = cp.tile([P, KT, H], f32)    # 1/||fk_raw|| (fv carries inv_sv)
        nc.scalar.activation(invn_k[:], ksq[:], ACTF.Abs_reciprocal_sqrt)
        cmk = cp.tile([P, KT, H], fp16)      # ksum/64
        nc.vector.tensor_scalar_mul(cmk[:], ksum[:], 1.0 / HD)
        # kvcol = (ksq - ksum^2/64) * inv_sk^2 / 63  (inv_sv lives in fv_all)
        t1 = cp.tile([P, KT, H], f32)
        nc.vector.tensor_mul(t1[:], ksum[:], ksum[:])
        nc.vector.scalar_tensor_tensor(t1[:], t1[:], -1.0 / HD, ksq[:],
                                       op0=ALU.mult, op1=ALU.add)
        t2 = cp.tile([P, KT], f32)
        nc.vector.tensor_mul(t2[:], inv_sk[:], inv_sk[:])
        nc.vector.tensor_scalar_mul(t1[:], t1[:], 1.0 / (HD - 1))
        kvcol = cp.tile([P, KT, H], fp16)
        nc.vector.tensor_tensor(kvcol[:], t1[:],
                                t2[:].unsqueeze(2).broadcast_to((P, KT, H)),
                                op=ALU.mult)

        # ---- batched U_k build ----
        fk_v = fk_all[:].rearrange("p t (h d) -> p t h d", h=H)
        nc.vector.tensor_tensor(
            uk_all[:, :, :, 0, :], fk_v,
            invn_k[:].unsqueeze(3).broadcast_to((P, KT, H, HD)), op=ALU.mult)
        nc.gpsimd.tensor_tensor(
            uk_all[:, :, :, 1, :], fk_v,
            cmk[:].unsqueeze(3).broadcast_to((P, KT, H, HD)), op=ALU.subtract)
        nc.vector.tensor_tensor(
            uk_all[:, :, :, 1, :], uk_all[:, :, :, 1, :],
            inv_sk[:].unsqueeze(2).unsqueeze(3).broadcast_to((P, KT, H, HD)),
            op=ALU.mult)

        # ---- per-head summary matrices ----
        psm = psM.tile([P, 512], f32, tag="pm")
        for h in range(H):
            for t in range(KT):
                nc.tensor.matmul(
                    psm[:, h * HD:(h + 1) * HD],
                    uk_all[:, t, h, :, :],
                    fv_all[:, t, h * HD:(h + 1) * HD],
                    start=(t == 0), stop=(t == KT - 1))
        psm3 = psR.tile([P, 512], f32, tag="pr")
        for t in range(KT):
            nc.tensor.matmul(psm3[0:H, :], kvcol[:, t, :], fv_all[:, t, :],
                             start=(t == 0), stop=(t == KT - 1))

        # B and RW = (var/d)blockdiag(m3) @ woT are both LINEAR in the partial
        # summaries, so they are computed on the partials and the AllReduce
        # carries the finished [B; RW] - nothing to compute after the reduce.
        B_part = cp.tile([P, 512], fp16)
        nc.scalar.activation(B_part[0:HD, :], psm[0:HD, :], ACTF.Copy, scale=cos_w)
        nc.scalar.activation(B_part[HD:P, :], psm[HD:P, :], ACTF.Copy, scale=c_cov)
        R_part = cp.tile([H, 512], fp16)
        nc.vector.scalar_tensor_tensor(R_part[:], psm3[0:H, :], c_var, bdmask[:],
                                       op0=ALU.mult, op1=ALU.mult)
        RT_sb = cp.tile([P, 4, H], fp16)
        for c in range(4):
            pt = psT.tile([P, P], fp16, tag="ptx")
            nc.tensor.transpose(pt[0:P, 0:H], R_part[:, c * P:(c + 1) * P],
                                ident16[0:H, 0:H])
            nc.scalar.copy(RT_sb[:, c, :], pt[0:P, 0:H])
        psrw = psR.tile([P, 512], f32, tag="pr")
        for c in range(4):
            nc.tensor.matmul(psrw[0:H, :], RT_sb[:, c, :], woT_sb[:, c, :],
                             start=(c == 0), stop=(c == 3))
        RW_part = cp.tile([H, 512], fp16)
        nc.scalar.copy(RW_part[:], psrw[0:H, :])

        cc_in = nc.dram_tensor("cc_in", [P + H, 512], fp16)
        cc_out = nc.dram_tensor("cc_out", [P + H, 512], fp16)
        nc.sync.dma_start(cc_in[0:P, :], B_part[:])
        nc.sync.dma_start(cc_in[P:P + H, :], RW_part[:])
        nc.gpsimd.collective_compute(
            "AllReduce", ALU.add,
            ins=[cc_in[:]], outs=[cc_out[:]],
            replica_groups=[[0, 1], [2, 3], [4, 5], [6, 7]])
        for t in range(QT // 2, QT):
            proj_tile(xq, t, st2_q, fq_all, (qsum, qsq), True)

        B_sb = cp.tile([P, 512], fp16)
        nc.sync.dma_start(B_sb[:], cc_out[0:P, :])
        RW_sb = cp.tile([H, 512], fp16)
        nc.sync.dma_start(RW_sb[:], cc_out[P:P + H, :])

        # ---- batched q-side derivations + U_q + transposed views ----
        inv_sq_ = cp.tile([P, QT], f32)
        nc.scalar.activation(inv_sq_[:], st2_q[:, :, 1], ACTF.Abs_reciprocal_sqrt,
                             bias=eps_b[:])
        invn_q = cp.tile([P, QT, H], f32)
        nc.scalar.activation(invn_q[:], qsq[:], ACTF.Abs_reciprocal_sqrt)
        # qvar = (qsq - qsum^2/64) * inv_sq^2 / 63  (batched scalars)
        t3 = cp.tile([P, QT, H], f32)
        nc.vector.tensor_mul(t3[:], qsum[:], qsum[:])
        nc.vector.scalar_tensor_tensor(t3[:], t3[:], -1.0 / HD, qsq[:],
                                       op0=ALU.mult, op1=ALU.add)
        t4 = cp.tile([P, QT], f32)
        nc.vector.tensor_mul(t4[:], inv_sq_[:], inv_sq_[:])
        nc.vector.tensor_scalar_mul(t3[:], t3[:], 1.0 / (HD - 1))
        qv_all = cp.tile([P, QT, H], fp16)
        nc.vector.tensor_tensor(qv_all[:], t3[:],
                                t4[:].unsqueeze(2).broadcast_to((P, QT, H)),
                                op=ALU.mult)

        # per-tile U_q build + transposes: the serialized uqT DMA-transposes
        # start as soon as tile 0's slice is built instead of after the batch
        fq_v = fq_all[:].rearrange("p t (h d) -> p t h d", h=H)
        uqTs, qvTs = [], []
        for t in range(QT):
            nc.vector.tensor_tensor(
                uq_all[:, t, :, 0, :], fq_v[:, t],
                invn_q[:, t, :].unsqueeze(2).broadcast_to((P, H, HD)),
                op=ALU.mult)
            nc.vector.tensor_scalar_mul(
                uq_all[:, t, :, 1, :], fq_v[:, t], inv_sq_[:, t:t + 1])
            uqT = uqp.tile([P, H, P], fp16, tag="uqT", name=f"uqT{t}")
            nc.sync.dma_start_transpose(
                uqT[:],
                uq_all[:, t, :, :, :].rearrange("p h two d -> p (h two d)"))
            pq = psT.tile([P, P], fp16, tag="ptx", name=f"pq{t}")
            nc.tensor.transpose(pq[0:H, :], qv_all[:, t, :], ident16[:])
            qvT = sp.tile([H, P], fp16, tag="qvT", name=f"qvT{t}")
            nc.scalar.copy(qvT[:], pq[0:H, :])
            uqTs.append(uqT)
            qvTs.append(qvT)

        # ---- attention + output projection per q tile ----
        # attn output is produced TRANSPOSED: psaT[p, c*128+tok] = attn[tok, 128c+p]
        # (B stationary, uqT moving; head h lands at partitions 64*(h%2),
        # free cols 128*(h//2)) so the out-proj lhsT needs no extra transposes.
        for t in range(QT):
            psa = psF.tile([P, D], f32, tag="pf")
            for h in range(H):
                po, co = HD * (h % 2), P * (h // 2)
                nc.tensor.matmul(psa[po:po + HD, co:co + P],
                                 B_sb[:, h * HD:(h + 1) * HD], uqTs[t][:, h, :],
                                 start=True, stop=True)
            catT = evp.tile([P, D], fp16, tag="at_sb")
            if t % 2 == 0:
                nc.scalar.copy(catT[:], psa[:])
            else:
                nc.vector.tensor_copy(catT[:], psa[:])

            pso = psF.tile([P, D], f32, tag="pf")
            for c in range(4):
                nc.tensor.matmul(pso[:], catT[:, c * P:(c + 1) * P],
                                 woT_sb[:, c, :], start=(c == 0), stop=False)
            nc.tensor.matmul(pso[:], qvTs[t][:], RW_sb[:], start=False, stop=True)
            o_sb = evp.tile([P, D], f32, tag="o_sb")
            if t % 2 == 0:
                nc.vector.tensor_copy(o_sb[:], pso[:])
            else:
                nc.scalar.copy(o_sb[:], pso[:])
            nc.sync.dma_start(out_d[t * P:(t + 1) * P, :], o_sb[:])

    nc.compile()
    return nc


_NC_CACHE = {}


def kernel(q, k, v, ln_gamma, ln_beta, w_in, w_out, b_out, cov_w_raw, var_w_raw):
    q = np.ascontiguousarray(np.asarray(q, dtype=np.float32))
    k = np.ascontiguousarray(np.asarray(k, dtype=np.float32))
    v = np.ascontiguousarray(np.asarray(v, dtype=np.float32))
    ln_gamma = np.asarray(ln_gamma, dtype=np.float32)
    ln_beta = np.asarray(ln_beta, dtype=np.float32)
    w_in = np.asarray(w_in, dtype=np.float32)
    w_out = np.asarray(w_out, dtype=np.float32)
    b_out = np.asarray(b_out, dtype=np.float32)
    assert np.all(ln_beta == 0.0), "kernel assumes LayerNorm beta == 0"
    assert np.all(b_out == 0.0), "kernel assumes b_out == 0"

    def sigmoid(x):
        return 1.0 / (1.0 + np.exp(-float(x)))

    cov_w = sigmoid(cov_w_raw)
    var_w = sigmoid(var_w_raw)
    cos_w = 1.0 - cov_w - var_w

    wg = w_in * ln_gamma[None, :]          # [inner, d]
    wgT = np.ascontiguousarray(wg.T)       # [d, inner]
    woT = np.ascontiguousarray(-w_out.T)   # negated: cancels the (mean-x) flip

    key = (round(float(cos_w), 8), round(float(cov_w), 8), round(float(var_w), 8))
    if key not in _NC_CACHE:
        _NC_CACHE[key] = build_kernel(cos_w, cov_w, var_w)
    nc = _NC_CACHE[key]

    in_maps = []
    for c in range(NCORES):
        g, s = c // 2, c % 2
        in_maps.append({
            "xq": np.ascontiguousarray(q[g, s * TQ:(s + 1) * TQ, :]),
            "xk": np.ascontiguousarray(k[g, s * TK:(s + 1) * TK, :]),
            "xv": np.ascontiguousarray(v[g, s * TK:(s + 1) * TK, :]),
            "wgT": wgT,
            "woT": woT,
        })
    res = run_bass_kernel_spmd(nc, in_maps, core_ids=list(range(NCORES))).results

    out = np.empty((QG, N, D), dtype=np.float32)
    for c in range(NCORES):
        g, s = c // 2, c % 2
        out[g, s * TQ:(s + 1) * TQ, :] = res[c]["out"]
    return out



# revision 6
# speedup vs baseline: 1.3666x; 1.3666x over previous
"""Trainium2 Bass kernel for nn_Attention_69801808495308.

Softmax-free attention: attn = cos_w*cossim + cov_w*cov/d + var_w*varprod/d is
linear in k-side summaries, so attn @ f_v reassociates into per-head 64x64
matrices (linear-attention trick) - no NxN score matrix is materialized.

Per (group g, head h), with fk/fv/fq the projected features:
  M1 = (fk/||fk||)^T fv_true        [64,64]
  M2 = (fk - mean(fk))^T fv_true    [64,64]   (columns sum to 0 -> q-centering free)
  m3 = kvar^T fv_true               [64]
  out = [cos_w*(fq/||fq||)@M1 + (cov_w/d)*fq_true@M2] @ woT
        + qvar @ RW + b_out,   RW = (var_w/d)*blockdiag(m3) @ woT

Sharding: 8 cores = (group g in 0..3) x (row half s in 0..1); q and k/v rows
are split across the pair. B and RW are linear in the k/v summaries, so each
core computes them on its partial sums and a single pair-wise fp16 AllReduce
(~140KB) finishes them - the only cross-core communication; its latency hides
under the entire q-side pipeline which is emitted after the collective.

v2 restructure (trace-driven):
- All inputs host-cast to fp16; x arrives as 6 big HWDGE DMAs (no SWDGE
  casts, half the HBM bytes); output written fp16 and up-cast on host.
- PE warm-up burst at t=0 so the HAM clock-gate reaches 8/8 before the real
  matmul stream starts, and the stream stays dense enough to keep it there.
- k/v pipeline: per-tile U_k builds on DVE overlap the next tile's PE work;
  the per-head M-matrix accumulation chain for tile t issues after tile
  t+1's projection so PE never waits on DVE.
- AllReduce is issued BEFORE any q-side work; the whole q pipeline plus the
  U_q builds fill its latency window.
- Attention matmuls batched: one matmul per (head, 4-tile quad) with the
  4 uqT tiles as a single 512-wide moving operand.
"""
import numpy as np
from contextlib import ExitStack

import concourse.bass as bass
from concourse import bacc
import concourse.tile as tile
import concourse.mybir as mybir
from concourse.bass_utils import run_bass_kernel_spmd
from concourse.masks import make_identity

f32 = mybir.dt.float32
fp16 = mybir.dt.float16
ALU = mybir.AluOpType
ACTF = mybir.ActivationFunctionType
AXX = mybir.AxisListType.X

QG, N, D = 4, 2048, 512
H, HD = 8, 64
P = 128
LN_EPS = 1e-5
TQ, TK = N // 2, N // 2
QT, KT = TQ // P, TK // P
NCORES = 8


def build_kernel(cos_w, cov_w, var_w):
    c_cov = cov_w / HD
    c_var = var_w / HD

    nc = bacc.Bacc("TRN2", target_bir_lowering=False, debug=False,
                   num_devices=NCORES)
    xq_d = nc.declare_dram_parameter("xq", [TQ, D], fp16, isOutput=False)
    xk_d = nc.declare_dram_parameter("xk", [TK, D], fp16, isOutput=False)
    xv_d = nc.declare_dram_parameter("xv", [TK, D], fp16, isOutput=False)
    wgT_d = nc.declare_dram_parameter("wgT", [D, D], fp16, isOutput=False)
    woT_d = nc.declare_dram_parameter("woT", [D, D], fp16, isOutput=False)
    out_d = nc.declare_dram_parameter("out", [TQ, D], fp16, isOutput=True)

    with tile.TileContext(nc) as tc, ExitStack() as ctx:
        cp = ctx.enter_context(tc.tile_pool(name="cp", bufs=1))
        slp = ctx.enter_context(tc.tile_pool(name="slp", bufs=4))
        sp = ctx.enter_context(tc.tile_pool(name="sp", bufs=6))
        evp = ctx.enter_context(tc.tile_pool(name="evp", bufs=3))

        # ---- constants ----
        ident16 = cp.tile([P, P], fp16)
        eps_b = cp.tile([P, 1], f32)
        nc.vector.memset(eps_b[:], LN_EPS)

        # ---- persistent state (all fp16 unless noted) ----
        xk_all = cp.tile([P, KT, D], fp16)     # raw k rows, token-major
        xv_all = cp.tile([P, KT, D], fp16)
        xq_all = cp.tile([P, QT, D], fp16)
        wgT_sb = cp.tile([P, 4, D], fp16)
        woT_sb = cp.tile([P, 4, D], fp16)
        fk_all = cp.tile([P, KT, D], fp16)     # projected (PSUM units)
        fv_all = cp.tile([P, KT, D], fp16)     # projected * inv_sigma_v
        fq_all = cp.tile([P, QT, D], fp16)
        uk_all = cp.tile([P, KT, H, 2, HD], fp16)
        uq_all = cp.tile([P, QT, H, 2, HD], fp16)
        uqT_all = cp.tile([P, QT, H, P], fp16)  # DMA-transposed uq, t-major
        st2_k = cp.tile([P, KT, 2], f32)
        st2_v = cp.tile([P, KT, 2], f32)
        st2_q = cp.tile([P, QT, 2], f32)
        ksum = cp.tile([P, KT, H], fp16)
        ksq = cp.tile([P, KT, H], fp16)
        qsum = cp.tile([P, QT, H], fp16)
        qsq = cp.tile([P, QT, H], fp16)
        inv_sk = cp.tile([P, KT], f32)
        inv_sq_ = cp.tile([P, QT], f32)
        qv_all = cp.tile([P, QT, H], fp16)

        # ---- input DMAs lead the SP ring; weight for proj comes first ----
        nc.sync.dma_start(wgT_sb[:], wgT_d[:].rearrange("(c p) n -> p c n", p=P))
        nc.sync.dma_start(
            xk_all[:, 0:KT // 2, :],
            xk_d[0:TK // 2, :].rearrange("(t p) d -> p t d", p=P))
        nc.sync.dma_start(
            xv_all[:, 0:KT // 2, :],
            xv_d[0:TK // 2, :].rearrange("(t p) d -> p t d", p=P))
        nc.sync.dma_start(
            xk_all[:, KT // 2:KT, :],
            xk_d[TK // 2:TK, :].rearrange("(t p) d -> p t d", p=P))
        nc.sync.dma_start(
            xv_all[:, KT // 2:KT, :],
            xv_d[TK // 2:TK, :].rearrange("(t p) d -> p t d", p=P))
        nc.sync.dma_start(woT_sb[:], woT_d[:].rearrange("(c p) n -> p c n", p=P))
        nc.sync.dma_start(
            xq_all[:, 0:QT // 2, :],
            xq_d[0:TQ // 2, :].rearrange("(t p) d -> p t d", p=P))
        nc.sync.dma_start(
            xq_all[:, QT // 2:QT, :],
            xq_d[TQ // 2:TQ, :].rearrange("(t p) d -> p t d", p=P))

        make_identity(nc, ident16)
        bdmask = cp.tile([H, 512], f32)
        nc.gpsimd.memset(bdmask[:], 0.0)
        nc.gpsimd.affine_select(
            out=bdmask[:].rearrange("p (b d) -> p b d", b=H),
            in_=bdmask[:].rearrange("p (b d) -> p b d", b=H),
            compare_op=ALU.not_equal, fill=1.0, base=0,
            pattern=[[-1, H], [0, HD]], channel_multiplier=1)

        psF = ctx.enter_context(tc.tile_pool(name="psF", bufs=2, space="PSUM"))
        psT = ctx.enter_context(tc.tile_pool(name="psT", bufs=2, space="PSUM"))
        psM = ctx.enter_context(tc.tile_pool(name="psM", bufs=1, space="PSUM"))
        psR = ctx.enter_context(tc.tile_pool(name="psR", bufs=1, space="PSUM"))
        psA = ctx.enter_context(tc.tile_pool(name="psA", bufs=2, space="PSUM"))

        # ---- PE warm-up: ~3.5us of back-to-back matmuls while DMAs land,
        # so the HAM clock-gate is at 8/8 when the real stream begins ----
        ps_w = psF.tile([P, D], f32, tag="pf", name="warm")
        for i in range(30):
            nc.tensor.matmul(ps_w[:, 0:P], ident16[:], ident16[:],
                             start=True, stop=True)

        def stage1(x_all, t, st2_all):
            """LN stats for resident tile t, then center in place as
            (mean - x) on ACT: global sign flip cancelled by negating w_out
            on the host."""
            xt = x_all[:, t, :]
            st6 = sp.tile([P, 6], f32, tag="st6")
            nc.vector.bn_stats(st6[:], xt)
            nc.vector.bn_aggr(st2_all[:, t, :], st6[:])
            nc.scalar.activation(xt, xt, ACTF.Identity,
                                 bias=st2_all[:, t, 0:1], scale=-1.0)
            return xt

        def stage2(xt, t, f_dst, head_stats, pe_transpose, evac_scale=None):
            """Transpose (PE or DMA), 4-matmul projection chain, ACT evac
            (optionally scaled); per-head sums on DVE."""
            slab = slp.tile([P, 4, P], fp16, tag="slab")
            if pe_transpose:
                for c in range(4):
                    pt = psT.tile([P, P], fp16, tag="ptx")
                    nc.tensor.transpose(pt[:], xt[:, c * P:(c + 1) * P], ident16[:])
                    if c % 2 == 0:
                        nc.scalar.copy(slab[:, c, :], pt[:])
                    else:
                        nc.vector.tensor_copy(slab[:, c, :], pt[:])
            else:
                nc.sync.dma_start_transpose(slab[:], xt)

            psf = psF.tile([P, D], f32, tag="pf")
            for c in range(4):
                nc.tensor.matmul(psf[:], slab[:, c, :], wgT_sb[:, c, :],
                                 start=(c == 0), stop=(c == 3))
            if evac_scale is not None:
                nc.scalar.activation(f_dst[:, t, :], psf[:], ACTF.Copy,
                                     scale=evac_scale)
            else:
                nc.scalar.copy(f_dst[:, t, :], psf[:])
            if head_stats is not None:
                hsum, hsq = head_stats
                fv_ = f_dst[:, t, :].rearrange("p (h d) -> p h d", h=H)
                with nc.allow_low_precision(reason="head sums fit fp16"):
                    nc.vector.reduce_sum(hsum[:, t, :], fv_, axis=AXX)
                    sq = evp.tile([P, D], fp16, tag="sq")
                    nc.vector.tensor_mul(sq[:], f_dst[:, t, :], f_dst[:, t, :])
                    nc.vector.reduce_sum(
                        hsq[:, t, :], sq[:].rearrange("p (h d) -> p h d", h=H),
                        axis=AXX)

        def uk_build(t):
            """Per-tile U_k slices on DVE/ACT so they overlap PE's next tile."""
            nc.scalar.activation(inv_sk[:, t:t + 1], st2_k[:, t, 1:2],
                                 ACTF.Abs_reciprocal_sqrt, bias=eps_b[:])
            invn = sp.tile([P, H], f32, tag="invn")
            nc.scalar.activation(invn[:], ksq[:, t, :], ACTF.Abs_reciprocal_sqrt)
            fk_t = fk_all[:, t, :].rearrange("p (h d) -> p h d", h=H)
            nc.vector.tensor_tensor(
                uk_all[:, t, :, 0, :], fk_t,
                invn[:].unsqueeze(2).broadcast_to((P, H, HD)), op=ALU.mult)
            cmk = sp.tile([P, H], fp16, tag="cmk")
            nc.vector.tensor_scalar_mul(cmk[:], ksum[:, t, :], 1.0 / HD)
            nc.vector.tensor_tensor(
                uk_all[:, t, :, 1, :], fk_t,
                cmk[:].unsqueeze(2).broadcast_to((P, H, HD)), op=ALU.subtract)
            nc.vector.tensor_tensor(
                uk_all[:, t, :, 1, :], uk_all[:, t, :, 1, :],
                inv_sk[:, t:t + 1].unsqueeze(2).broadcast_to((P, H, HD)),
                op=ALU.mult)

        # M-matrix accumulation: single PSUM bank, one has_written clear at
        # the very first matmul; every later write either overwrites a
        # cleared region (first t for that head) or accumulates (t>0).
        psm = psM.tile([P, 512], f32, tag="pm")

        def m_chain(t):
            for h in range(H):
                nc.tensor.matmul(
                    psm[:, h * HD:(h + 1) * HD],
                    uk_all[:, t, h, :, :],
                    fv_all[:, t, h * HD:(h + 1) * HD],
                    start=(t == 0 and h == 0), stop=(t == KT - 1))

        # ---- k/v pipeline: PE stream stays 1 tile ahead of the M-chain ----
        for t in range(KT):
            xv_t = stage1(xv_all, t, st2_v)
            xk_t = stage1(xk_all, t, st2_k)
            inv_sv_t = sp.tile([P, 1], f32, tag="invsv")
            nc.scalar.activation(inv_sv_t[:], st2_v[:, t, 1:2],
                                 ACTF.Abs_reciprocal_sqrt, bias=eps_b[:])
            stage2(xv_t, t, fv_all, None, False, evac_scale=inv_sv_t[:])
            stage2(xk_t, t, fk_all, (ksum, ksq), True)
            uk_build(t)
            if t >= 1:
                m_chain(t - 1)
        m_chain(KT - 1)

        # ---- batched kvcol + m3 chain ----
        t1 = cp.tile([P, KT, H], f32)
        nc.vector.tensor_mul(t1[:], ksum[:], ksum[:])
        nc.vector.scalar_tensor_tensor(t1[:], t1[:], -1.0 / HD, ksq[:],
                                       op0=ALU.mult, op1=ALU.add)
        t2 = cp.tile([P, KT], f32)
        nc.vector.tensor_mul(t2[:], inv_sk[:], inv_sk[:])
        nc.vector.tensor_scalar_mul(t1[:], t1[:], 1.0 / (HD - 1))
        kvcol = cp.tile([P, KT, H], fp16)
        nc.vector.tensor_tensor(kvcol[:], t1[:],
                                t2[:].unsqueeze(2).broadcast_to((P, KT, H)),
                                op=ALU.mult)
        psm3 = psR.tile([P, 512], f32, tag="pr")
        for t in range(KT):
            nc.tensor.matmul(psm3[0:H, :], kvcol[:, t, :], fv_all[:, t, :],
                             start=(t == 0), stop=(t == KT - 1))

        # ---- B, RW on the partial sums; AllReduce carries finished values ----
        B_part = cp.tile([P, 512], fp16)
        nc.scalar.activation(B_part[0:HD, :], psm[0:HD, :], ACTF.Copy, scale=cos_w)
        nc.scalar.activation(B_part[HD:P, :], psm[HD:P, :], ACTF.Copy, scale=c_cov)
        R_part = cp.tile([H, 512], fp16)
        nc.vector.scalar_tensor_tensor(R_part[:], psm3[0:H, :], c_var, bdmask[:],
                                       op0=ALU.mult, op1=ALU.mult)
        RT_sb = cp.tile([P, 4, H], fp16)
        for c in range(4):
            pt = psT.tile([P, P], fp16, tag="ptx")
            nc.tensor.transpose(pt[0:P, 0:H], R_part[:, c * P:(c + 1) * P],
                                ident16[0:H, 0:H])
            nc.scalar.copy(RT_sb[:, c, :], pt[0:P, 0:H])
        psrw = psR.tile([P, 512], f32, tag="pr")
        for c in range(4):
            nc.tensor.matmul(psrw[0:H, :], RT_sb[:, c, :], woT_sb[:, c, :],
                             start=(c == 0), stop=(c == 3))
        RW_part = cp.tile([H, 512], fp16)
        nc.scalar.copy(RW_part[:], psrw[0:H, :])

        cc_in = nc.dram_tensor("cc_in", [P + H, 512], fp16)
        cc_out = nc.dram_tensor("cc_out", [P + H, 512], fp16)
        nc.sync.dma_start(cc_in[0:P, :], B_part[:])
        nc.sync.dma_start(cc_in[P:P + H, :], RW_part[:])
        nc.gpsimd.collective_compute(
            "AllReduce", ALU.add,
            ins=[cc_in[:]], outs=[cc_out[:]],
            replica_groups=[[0, 1], [2, 3], [4, 5], [6, 7]])

        # ---- entire q pipeline fills the collective's latency window ----
        for t in range(QT):
            xq_t = stage1(xq_all, t, st2_q)
            stage2(xq_t, t, fq_all, (qsum, qsq), True)
            nc.scalar.activation(inv_sq_[:, t:t + 1], st2_q[:, t, 1:2],
                                 ACTF.Abs_reciprocal_sqrt, bias=eps_b[:])
            invnq = sp.tile([P, H], f32, tag="invn")
            nc.scalar.activation(invnq[:], qsq[:, t, :], ACTF.Abs_reciprocal_sqrt)
            fq_t = fq_all[:, t, :].rearrange("p (h d) -> p h d", h=H)
            nc.vector.tensor_tensor(
                uq_all[:, t, :, 0, :], fq_t,
                invnq[:].unsqueeze(2).broadcast_to((P, H, HD)), op=ALU.mult)
            nc.vector.tensor_scalar_mul(
                uq_all[:, t, :, 1, :], fq_t, inv_sq_[:, t:t + 1])
            nc.sync.dma_start_transpose(
                uqT_all[:, t, :, :],
                uq_all[:, t, :, :, :].rearrange("p h two d -> p (h two d)"))

        # qvar = (qsq - qsum^2/64) * inv_sq^2 / 63  (batched)
        t3 = cp.tile([P, QT, H], f32)
        nc.vector.tensor_mul(t3[:], qsum[:], qsum[:])
        nc.vector.scalar_tensor_tensor(t3[:], t3[:], -1.0 / HD, qsq[:],
                                       op0=ALU.mult, op1=ALU.add)
        t4 = cp.tile([P, QT], f32)
        nc.vector.tensor_mul(t4[:], inv_sq_[:], inv_sq_[:])
        nc.vector.tensor_scalar_mul(t3[:], t3[:], 1.0 / (HD - 1))
        nc.vector.tensor_tensor(qv_all[:], t3[:],
                                t4[:].unsqueeze(2).broadcast_to((P, QT, H)),
                                op=ALU.mult)

        # per-tile qv transposes (small; PE has headroom here)
        qvT_all = cp.tile([H, QT, P], fp16)
        for t in range(QT):
            pq = psT.tile([P, P], fp16, tag="ptx", name=f"pq{t}")
            nc.tensor.transpose(pq[0:H, :], qv_all[:, t, :], ident16[:])
            nc.scalar.copy(qvT_all[:, t, :], pq[0:H, :])

        B_sb = cp.tile([P, 512], fp16)
        nc.sync.dma_start(B_sb[:], cc_out[0:P, :])
        RW_sb = cp.tile([H, 512], fp16)
        nc.sync.dma_start(RW_sb[:], cc_out[P:P + H, :])

        # ---- attention: per (head, 4-tile quad) batched matmuls.
        # psa_pair holds head-pair hp transposed: [128 = (h%2)*64+d,
        # 512 = 4 tiles x 128 tok]; after evac it IS the out-proj lhsT chunk.
        catT_all = cp.tile([P, 4, 512], fp16)
        for q in range(2):
            for hp in range(4):
                psa = psF.tile([P, 512], f32, tag="pf", name=f"psa{q}_{hp}")
                for j in range(2):
                    h = 2 * hp + j
                    nc.tensor.matmul(
                        psa[64 * j:64 * j + 64, :],
                        B_sb[:, h * HD:(h + 1) * HD],
                        uqT_all[:, 4 * q:4 * q + 4, h, :],
                        start=True, stop=True)
                if hp % 2 == 0:
                    nc.scalar.copy(catT_all[:, hp, :], psa[:])
                else:
                    nc.vector.tensor_copy(catT_all[:, hp, :], psa[:])
            for tr in range(4):
                t = 4 * q + tr
                pso = psA.tile([P, D], f32, tag="po")
                for c in range(4):
                    nc.tensor.matmul(
                        pso[:], catT_all[:, c, tr * P:(tr + 1) * P],
                        woT_sb[:, c, :], start=(c == 0), stop=False)
                nc.tensor.matmul(pso[:], qvT_all[:, t, :], RW_sb[:],
                                 start=False, stop=True)
                o_sb = evp.tile([P, D], fp16, tag="o_sb")
                if tr % 2 == 0:
                    nc.vector.tensor_copy(o_sb[:], pso[:])
                else:
                    nc.scalar.copy(o_sb[:], pso[:])
                nc.sync.dma_start(out_d[t * P:(t + 1) * P, :], o_sb[:])

    nc.compile()
    return nc


_NC_CACHE = {}


def kernel(q, k, v, ln_gamma, ln_beta, w_in, w_out, b_out, cov_w_raw, var_w_raw):
    q = np.asarray(q, dtype=np.float32)
    k = np.asarray(k, dtype=np.float32)
    v = np.asarray(v, dtype=np.float32)
    ln_gamma = np.asarray(ln_gamma, dtype=np.float32)
    ln_beta = np.asarray(ln_beta, dtype=np.float32)
    w_in = np.asarray(w_in, dtype=np.float32)
    w_out = np.asarray(w_out, dtype=np.float32)
    b_out = np.asarray(b_out, dtype=np.float32)
    assert np.all(ln_beta == 0.0), "kernel assumes LayerNorm beta == 0"
    assert np.all(b_out == 0.0), "kernel assumes b_out == 0"

    def sigmoid(x):
        return 1.0 / (1.0 + np.exp(-float(x)))

    cov_w = sigmoid(cov_w_raw)
    var_w = sigmoid(var_w_raw)
    cos_w = 1.0 - cov_w - var_w

    wg = w_in * ln_gamma[None, :]          # [inner, d]
    wgT = np.ascontiguousarray(wg.T).astype(np.float16)       # [d, inner]
    woT = np.ascontiguousarray(-w_out.T).astype(np.float16)   # negated flip

    q16 = q.astype(np.float16)
    k16 = k.astype(np.float16)
    v16 = v.astype(np.float16)

    key = (round(float(cos_w), 8), round(float(cov_w), 8), round(float(var_w), 8))
    if key not in _NC_CACHE:
        _NC_CACHE[key] = build_kernel(cos_w, cov_w, var_w)
    nc = _NC_CACHE[key]

    in_maps = []
    for c in range(NCORES):
        g, s = c // 2, c % 2
        in_maps.append({
            "xq": np.ascontiguousarray(q16[g, s * TQ:(s + 1) * TQ, :]),
            "xk": np.ascontiguousarray(k16[g, s * TK:(s + 1) * TK, :]),
            "xv": np.ascontiguousarray(v16[g, s * TK:(s + 1) * TK, :]),
            "wgT": wgT,
            "woT": woT,
        })
    res = run_bass_kernel_spmd(nc, in_maps, core_ids=list(range(NCORES))).results

    out = np.empty((QG, N, D), dtype=np.float32)
    for c in range(NCORES):
        g, s = c // 2, c % 2
        out[g, s * TQ:(s + 1) * TQ, :] = res[c]["out"].astype(np.float32)
    return out
